# revision 14
# baseline (speedup 1.0000x reference)
"""DNPU layer (128 independent per-expert MLPs, batch 16384) on 8 trn2 cores.

Sharding: expert-parallel - core k owns experts 16k..16k+15 and the full
batch. Host-side prep folds the control electrodes and every bias into the
weight blocks (ones-row trick), transposes x into an electrode-major layout,
and zero-pads every matmul to K=128 x M=128.

The kernel is co-bound by PE stream time and the PSUM->SBUF relu drain on
ScalarE/VectorE (fp32 psum reads run at 1 elem/cycle/lane on both engines,
~1076/1211 ns per [*,1024] tile). Design:
  - fp16 operands (1 col/cycle PE stream like bf16, ~8x better mantissa);
  - batch pairs run in chunks of 4: the 8 matmuls of one weight block
    share one LDWEIGHTS, amortizing the ~150 ns weight-load exposure;
  - a globally staggered software pipeline (stages L0/L1/L2/L3+out at
    super-iteration lags 0/2/6/10/12) keeps the PE:relu work mix uniform
    so neither the PE nor the drain engines ever starve;
  - the output projection rides the layer-3 psum tiles: out[t,q]+4.0 is
    written by M=1 matmuls into psum partition 96 (unused above M=91) of
    the layer-3 tile of unit t+2 right before its relu, so the existing
    relu drains it for free (relu passes out+4 > 0 unchanged), and the
    result row is DMAd to DRAM straight out of the h tile. No dedicated
    out psum tile (-> 4 rotating layer slots = all 8 banks), no psum
    accumulation chain, no out-copy drain ops. Host subtracts the 4.0.
  - relus alternate ScalarE/VectorE on a period-13 pattern (7:6 ~ the
    1076:1211 ns rate ratio).
"""

import sys

if "/opt/trn_rl_repo" not in sys.path:
    sys.path.insert(0, "/opt/trn_rl_repo")

from contextlib import ExitStack

import numpy as np

import concourse.bass as bass
import concourse.mybir as mybir
import concourse.tile as tile
from concourse.bass import ds, ts

B = 16384  # batch
N = 128  # experts
I = 3  # data electrodes / expert
C = 4  # control electrodes / expert
H = 90  # hidden width
L = 3  # extra hidden layers
NCORES = 8
NLOC = N // NCORES  # 16 experts per core
F = 512  # matmul moving free dim (one fp32 psum bank)
PAIR = 2 * F  # pointwise tile width
CHUNK = 4  # batch pairs per weight load

BLK = 128  # padded K and M
W0_W = NLOC * BLK  # 2048
WH_W = L * NLOC * BLK  # 6144
WO_W = NLOC  # one [128, 1] w_out column per expert
WALL_W = W0_W + WH_W + WO_W
ORow = 96  # psum/h partition row carrying the ridden output
OFFSET = 4.0  # out bias shift so relu(out + OFFSET) == out + OFFSET

MM_DTYPE = "fp16"

# psum->sbuf relu engine pattern: ScalarE (0) ~1076 ns / [*,1024] fp32 psum
# tile, VectorE (1) ~1211 ns -> 7:6 split over period 13.
ENG_PAT = [0, 1, 0, 1, 0, 1, 0, 1, 0, 1, 0, 1, 0]
ENG_PERIOD = len(ENG_PAT)
H_BUFS = 52  # 4*13: h-tile WAR lands on the producing engine

LAG1, LAG2, LAG3 = 2, 4, 6
LAGO = LAG3 + 2  # out rides the L3 tiles of unit t+2


def build_nc(b=B, mm_dtype=None, h_bufs=H_BUFS):
    """Build the single-core Bass program (SPMD across cores via data)."""
    nchunk = b // (CHUNK * PAIR)
    NUNIT = nchunk * NLOC
    f32 = mybir.dt.float32
    mmdt = {
        "fp16": mybir.dt.float16,
        "bf16": mybir.dt.bfloat16,
        "f32r": mybir.dt.float32r,
    }[mm_dtype or MM_DTYPE]

    nc = bass.Bass("TRN2", target_bir_lowering=False, debug=False)
    xTr = nc.dram_tensor("xTr", [BLK, b], mmdt, kind="ExternalInput").ap()
    wall = nc.dram_tensor("wall", [128, WALL_W], mmdt, kind="ExternalInput").ap()
    # row CHUNK*n+q = expert n, pair q of each chunk; host de-interleaves
    outT = nc.dram_tensor(
        "outT", [CHUNK * NLOC, b // CHUNK], mmdt, kind="ExternalOutput"
    ).ap()

    from concourse.tile import add_dep_helper

    with ExitStack() as ctx:
        tc = ctx.enter_context(tile.TileContext(nc))
        wpool = ctx.enter_context(tc.tile_pool(name="w", bufs=1))
        xpool = ctx.enter_context(tc.tile_pool(name="x", bufs=2 * CHUNK))
        hpool = ctx.enter_context(tc.tile_pool(name="h", bufs=h_bufs))
        # 4 rotating layer psum slots (2 banks each) = all 8 PSUM banks
        pspool = ctx.enter_context(tc.tile_pool(name="ps", bufs=4, space="PSUM"))

        wall_sb = wpool.tile([128, WALL_W], mmdt)
        # Split the weight DMA per layer block: Tile's range tracking then
        # lets layer-l matmuls start as soon as their block has landed.
        cuts = [0, W0_W, W0_W + NLOC * BLK, W0_W + 2 * NLOC * BLK, WALL_W]
        for a, b_ in zip(cuts[:-1], cuts[1:]):
            dma_w = nc.sync.dma_start(wall_sb[:, a:b_], wall[:, a:b_])
            # PE NOP probe: absorbs the weight-DMA queue sem into PE's
            # observed clock (matmuls have a 1-sync-wait codegen budget).
            nop_w = nc.tensor.nop()
            add_dep_helper(nop_w.ins, dma_w.ins, reason="absorb wall dma wait")

        alloc_cnt = [0]

        def psum_alloc():
            c = alloc_cnt[0]
            t = pspool.tile([128, PAIR], f32, name="ps", tag="ps")
            alloc_cnt[0] += 1
            return t, c

        def pointwise(dst, src, relu, c):
            if ENG_PAT[c % ENG_PERIOD] == 0:
                func = (
                    mybir.ActivationFunctionType.Relu
                    if relu
                    else mybir.ActivationFunctionType.Identity
                )
                nc.scalar.activation(dst, src, func)
            elif relu:
                nc.vector.tensor_scalar_max(dst, src, 0.0)
            else:
                nc.vector.tensor_copy(dst, src)

        xts = {}  # chunk j -> [xt tiles]
        hs = {}  # (t, l) -> [h tiles per q]

        def prefetch_x(j):
            if j >= nchunk or j in xts:
                return
            tiles = []
            for q in range(CHUNK):
                xt = xpool.tile([BLK, PAIR], mmdt)
                dma_xt = nc.sync.dma_start(
                    xt[:], xTr[:, ds((CHUNK * j + q) * PAIR, PAIR)]
                )
                nop_x = nc.tensor.nop()
                add_dep_helper(nop_x.ins, dma_xt.ins, reason="absorb xt dma wait")
                tiles.append(xt)
            xts[j] = tiles

        def emit_layer(t, l, tout=None):
            """One weight block: CHUNK*2 layer matmuls + relu per pair; for
            l == L, unit `tout`'s output rows ride this unit's psum tiles.
            All same-weight matmuls stay back-to-back (layer block first,
            then all riders) so each weight block is loaded exactly once."""
            j, n = divmod(t, NLOC)
            if l == 0:
                lhsT = wall_sb[:, ds(n * BLK, BLK)]
            else:
                lhsT = wall_sb[:, ds(W0_W + ((l - 1) * NLOC + n) * BLK, BLK)]
            tiles = []
            for q in range(CHUNK):
                rhs = xts[j][q] if l == 0 else hs[(t, l - 1)][q]
                ps_t, c = psum_alloc()
                for v in range(2):
                    nc.tensor.matmul(ps_t[:, ts(v, F)], lhsT, rhs[:, ts(v, F)])
                tiles.append((ps_t, c))
            if tout is not None:
                jo, no = divmod(tout, NLOC)
                lhsT_o = wall_sb[:, ds(W0_W + WH_W + no, 1)]
                for q in range(CHUNK):
                    ps_t, _ = tiles[q]
                    h3 = hs[(tout, L)][q]
                    for v in range(2):
                        nc.tensor.matmul(
                            ps_t[ds(ORow, 1), ts(v, F)],
                            lhsT_o,
                            h3[:, ts(v, F)],
                            tile_position=(0, ORow),
                        )
            res = []
            for q in range(CHUNK):
                ps_t, c = tiles[q]
                ht = hpool.tile([BLK, PAIR], mmdt, tag="h")
                pointwise(ht[:], ps_t[:], True, c)
                res.append(ht)
                if tout is not None:
                    nc.sync.dma_start(
                        outT[ds(CHUNK * no + q, 1), ts(jo, PAIR)],
                        ht[ds(ORow, 1), :],
                    )
            hs[(t, l)] = res
            if l > 0:
                del hs[(t, l - 1)]
            if l == 0 and n == NLOC - 1:
                del xts[j]
            if tout is not None:
                del hs[(tout, L)]

        def emit_tail(touts):
            """Outputs of the last two units: one psum alloc per (unit, pair),
            rider row ORow each (32-aligned base required by walrus)."""
            for tout in touts:
                jo, no = divmod(tout, NLOC)
                lhsT_o = wall_sb[:, ds(W0_W + WH_W + no, 1)]
                for q in range(CHUNK):
                    ps_t, c = psum_alloc()
                    for v in range(2):
                        nc.tensor.matmul(
                            ps_t[ds(ORow, 1), ts(v, F)],
                            lhsT_o,
                            hs[(tout, L)][q][:, ts(v, F)],
                            tile_position=(0, ORow),
                        )
                    ht = hpool.tile([BLK, PAIR], mmdt, tag="h")
                    pointwise(ht[ds(ORow, 1), :], ps_t[ds(ORow, 1), :], True, c)
                    nc.sync.dma_start(
                        outT[ds(CHUNK * no + q, 1), ts(jo, PAIR)],
                        ht[ds(ORow, 1), :],
                    )
                del hs[(tout, L)]

        prefetch_x(0)
        for s in range(NUNIT + LAG3):
            if s % NLOC == NLOC - 2:
                prefetch_x(s // NLOC + 1)
            if s < NUNIT:
                emit_layer(s, 0)
            if LAG1 <= s < NUNIT + LAG1:
                emit_layer(s - LAG1, 1)
            if LAG2 <= s < NUNIT + LAG2:
                emit_layer(s - LAG2, 2)
            if LAG3 <= s < NUNIT + LAG3:
                t3 = s - LAG3
                tout = s - LAGO if s >= LAGO else None
                emit_layer(t3, L, tout=tout)
        emit_tail([NUNIT - 2, NUNIT - 1])
    return nc


def _split_excess_waits(bir_bytes: bytes) -> bytes:
    """BIR post-pass: walrus codegen allows at most ONE sync wait per engine
    instruction; hoist extra waits onto inserted no-update NoOps."""
    import json as _json

    d = _json.loads(bir_bytes)
    ctr = 0
    for fn in d.get("functions", []):
        for bb in fn.get("blocks", []):
            out = []
            for ins in bb.get("instructions", []):
                si = ins.get("sync_info") or {}
                ow = si.get("on_wait") or []
                if len(ow) > 1 and ins.get("engine"):
                    for w in ow[:-1]:
                        ctr += 1
                        out.append(
                            {
                                "debug": ins.get("debug", 0),
                                "engine": ins["engine"],
                                "ins": [],
                                "outs": [],
                                "name": f"WSPLIT-{ctr}",
                                "opcode": "NoOp",
                                "sync_info": {"on_update": [], "on_wait": [w]},
                            }
                        )
                    si["on_wait"] = [ow[-1]]
                out.append(ins)
            bb["instructions"] = out
    return _json.dumps(d).encode()


def install_wait_splitter():
    from concourse import bass2jax, bass_utils

    if getattr(bass_utils, "_ws_installed", False):
        return
    orig = bass_utils.compile_bir_kernel

    def patched(bir_json, tmpdir, neff_name="file.neff"):
        return orig(_split_excess_waits(bir_json), tmpdir, neff_name=neff_name)

    bass_utils.compile_bir_kernel = patched
    bass2jax.compile_bir_kernel = patched
    bass_utils._ws_installed = True


def prep_core_inputs(x, controls, W_in, b_in, W_hid, b_hid, W_out, b_out, b=B, mm_dtype=None):
    """Host-side fold + shard: list of per-core input dicts."""
    x = np.asarray(x, np.float32)
    controls = np.asarray(controls, np.float64)
    W_in = np.asarray(W_in, np.float64)
    b_in = np.asarray(b_in, np.float64)
    W_hid = np.asarray(W_hid, np.float32)
    b_hid = np.asarray(b_hid, np.float32)
    W_out = np.asarray(W_out, np.float32)
    b_out = np.asarray(b_out, np.float32)

    # controls fold: beff0[n] = controls[n] @ W_in[n, I:, :] + b_in[n]
    beff0 = (
        np.einsum("nc,nch->nh", controls, W_in[:, I:, :]) + b_in
    ).astype(np.float32)
    W_in_d = W_in[:, :I, :].astype(np.float32)  # [N, 3, H]

    # x transposed to electrode-major with ones rows, zero-padded to 128
    xT = np.ascontiguousarray(x.T).reshape(N, I, b)  # [N, 3, B]
    xTr = np.zeros((NCORES, BLK, b), np.float32)
    v = xTr[:, : 4 * NLOC, :].reshape(NCORES, NLOC, 4, b)
    v[:, :, :I, :] = xT.reshape(NCORES, NLOC, I, b)
    v[:, :, I, :] = 1.0

    npdt = mybir.dt.np(
        {"fp16": mybir.dt.float16, "bf16": mybir.dt.bfloat16, "f32r": mybir.dt.float32r}[
            mm_dtype or MM_DTYPE
        ]
    )
    in_maps = []
    for k in range(NCORES):
        g0 = k * NLOC
        wallm = np.zeros((128, WALL_W), np.float32)
        w0blk = wallm[:, :W0_W]
        whblk = wallm[:, W0_W : W0_W + WH_W]
        woblk = wallm[:, W0_W + WH_W :]
        for n in range(NLOC):
            g = g0 + n
            w0blk[4 * n : 4 * n + I, n * BLK : n * BLK + H] = W_in_d[g]
            w0blk[4 * n + I, n * BLK : n * BLK + H] = beff0[g]
            w0blk[4 * n + I, n * BLK + H] = 1.0
        for l in range(L):
            for n in range(NLOC):
                g = g0 + n
                base = (l * NLOC + n) * BLK
                whblk[:H, base : base + H] = W_hid[l, g]
                whblk[H, base : base + H] = b_hid[l, g]
                whblk[H, base + H] = 1.0
        woblk[:H, :] = W_out[g0 : g0 + NLOC].T
        woblk[H, :] = b_out[g0 : g0 + NLOC] + OFFSET
        in_maps.append(
            {
                "xTr": np.ascontiguousarray(xTr[k]).astype(npdt),
                "wall": wallm.astype(npdt),
            }
        )
    return in_maps


def run_sharded(inputs, b=B, mm_dtype=None, trace=False, **kw):
    """Build + run on the 8 cores; returns (out [b, N] fp32, BassKernelResults)."""
    from concourse import bass_utils

    install_wait_splitter()
    nc = build_nc(b=b, mm_dtype=mm_dtype)
    in_maps = prep_core_inputs(b=b, mm_dtype=mm_dtype, **inputs)
    res = bass_utils.run_bass_kernel_spmd(
        nc, in_maps, core_ids=list(range(NCORES)), trace=trace, **kw
    )
    out = np.empty((b, N), np.float32)
    nchunk = b // (CHUNK * PAIR)
    for k in range(NCORES):
        # outT row CHUNK*n+q, col j*PAIR+t  <->  batch (CHUNK*j+q)*PAIR+t
        o = res.results[k]["outT"].astype(np.float32) - OFFSET
        o = o.reshape(NLOC, CHUNK, nchunk, PAIR)
        o = o.transpose(2, 1, 3, 0).reshape(b, NLOC)
        out[:, k * NLOC : (k + 1) * NLOC] = o
    return out, res


def kernel(**inputs) -> np.ndarray:
    out, _ = run_sharded(inputs)
    return out


# revision 18
# speedup vs baseline: 1.0102x; 1.0102x over previous
"""DNPU layer (128 independent per-expert MLPs, batch 16384) on 8 trn2 cores.

Sharding: expert-parallel - core k owns experts 16k..16k+15 and the full
batch. Host-side prep folds the control electrodes and every bias into the
weight blocks (ones-row trick), transposes x into an electrode-major layout,
and zero-pads every matmul to K=128 x M=128.

The kernel is co-bound by PE stream time and the PSUM->SBUF relu drain on
ScalarE/VectorE (fp32 psum reads run at 1 elem/cycle/lane on both engines,
~1076/1211 ns per [*,1024] tile). Design:
  - fp16 operands (1 col/cycle PE stream like bf16, ~8x better mantissa);
  - batch pairs run in chunks of 4: the 8 matmuls of one weight block
    share one LDWEIGHTS, amortizing the ~150 ns weight-load exposure;
  - a globally staggered software pipeline (stages L0/L1/L2/L3+out at
    super-iteration lags 0/2/6/10/12) keeps the PE:relu work mix uniform
    so neither the PE nor the drain engines ever starve;
  - the output projection rides the layer-3 psum tiles: out[t,q]+4.0 is
    written by M=1 matmuls into psum partition 96 (unused above M=91) of
    the layer-3 tile of unit t+2 right before its relu, so the existing
    relu drains it for free (relu passes out+4 > 0 unchanged), and the
    result row is DMAd to DRAM straight out of the h tile. No dedicated
    out psum tile (-> 4 rotating layer slots = all 8 banks), no psum
    accumulation chain, no out-copy drain ops. Host subtracts the 4.0.
  - relus alternate ScalarE/VectorE on a period-13 pattern (7:6 ~ the
    1076:1211 ns rate ratio).
"""

import sys

if "/opt/trn_rl_repo" not in sys.path:
    sys.path.insert(0, "/opt/trn_rl_repo")

from contextlib import ExitStack

import numpy as np

import concourse.bass as bass
import concourse.mybir as mybir
import concourse.tile as tile
from concourse.bass import ds, ts

B = 16384  # batch
N = 128  # experts
I = 3  # data electrodes / expert
C = 4  # control electrodes / expert
H = 90  # hidden width
L = 3  # extra hidden layers
NCORES = 8
NLOC = N // NCORES  # 16 experts per core
F = 512  # matmul moving free dim (one fp32 psum bank)
PAIR = 2 * F  # pointwise tile width
CHUNK = 4  # batch pairs per weight load

BLK = 128  # padded K and M
W0_W = NLOC * BLK  # 2048
WH_W = L * NLOC * BLK  # 6144
WO_W = NLOC  # one [128, 1] w_out column per expert
WALL_W = W0_W + WH_W + WO_W
ORow = 96  # psum/h partition row carrying the ridden output
OFFSET = 4.0  # out bias shift so relu(out + OFFSET) == out + OFFSET

MM_DTYPE = "fp16"

# psum->sbuf relu engine pattern: ScalarE (0) ~1076 ns / [*,1024] fp32 psum
# tile, VectorE (1) ~1211 ns -> 7:6 split over period 13.
ENG_PAT = [0, 1, 0, 1, 0, 1, 0, 1, 0, 1, 0, 1, 0]
ENG_PERIOD = len(ENG_PAT)
H_BUFS = 52  # 4*13: h-tile WAR lands on the producing engine

LAG1, LAG2, LAG3 = 2, 4, 6
LAGO = LAG3 + 2  # out rides the L3 tiles of unit t+2


def build_nc(b=B, mm_dtype=None, h_bufs=H_BUFS):
    """Build the single-core Bass program (SPMD across cores via data)."""
    nchunk = b // (CHUNK * PAIR)
    NUNIT = nchunk * NLOC
    f32 = mybir.dt.float32
    mmdt = {
        "fp16": mybir.dt.float16,
        "bf16": mybir.dt.bfloat16,
        "f32r": mybir.dt.float32r,
    }[mm_dtype or MM_DTYPE]

    nc = bass.Bass("TRN2", target_bir_lowering=False, debug=False)
    xTr = nc.dram_tensor("xTr", [BLK, b], mmdt, kind="ExternalInput").ap()
    wall = nc.dram_tensor("wall", [128, WALL_W], mmdt, kind="ExternalInput").ap()
    # row CHUNK*n+q = expert n, pair q of each chunk; host de-interleaves
    outT = nc.dram_tensor(
        "outT", [CHUNK * NLOC, b // CHUNK], mmdt, kind="ExternalOutput"
    ).ap()

    from concourse.tile import add_dep_helper

    with ExitStack() as ctx:
        tc = ctx.enter_context(tile.TileContext(nc))
        wpool = ctx.enter_context(tc.tile_pool(name="w", bufs=1))
        xpool = ctx.enter_context(tc.tile_pool(name="x", bufs=2 * CHUNK))
        hpool = ctx.enter_context(tc.tile_pool(name="h", bufs=h_bufs))
        # 4 rotating layer psum slots (2 banks each) = all 8 PSUM banks
        pspool = ctx.enter_context(tc.tile_pool(name="ps", bufs=4, space="PSUM"))

        wall_sb = wpool.tile([128, WALL_W], mmdt)

        alloc_cnt = [0]

        def psum_alloc():
            c = alloc_cnt[0]
            t = pspool.tile([128, PAIR], f32, name="ps", tag="ps")
            alloc_cnt[0] += 1
            return t, c

        def pointwise(dst, src, relu, c):
            if ENG_PAT[c % ENG_PERIOD] == 0:
                func = (
                    mybir.ActivationFunctionType.Relu
                    if relu
                    else mybir.ActivationFunctionType.Identity
                )
                nc.scalar.activation(dst, src, func)
            elif relu:
                nc.vector.tensor_scalar_max(dst, src, 0.0)
            else:
                nc.vector.tensor_copy(dst, src)

        xts = {}  # chunk j -> [xt tiles]
        hs = {}  # (t, l) -> [h tiles per q]

        def dma_wall_block(a, b_):
            dma_w = nc.sync.dma_start(wall_sb[:, a:b_], wall[:, a:b_])
            # PE NOP probe: absorbs the weight-DMA queue sem into PE's
            # observed clock (matmuls have a 1-sync-wait codegen budget).
            nop_w = nc.tensor.nop()
            add_dep_helper(nop_w.ins, dma_w.ins, reason="absorb wall dma wait")

        def prefetch_x(j):
            if j >= nchunk or j in xts:
                return
            tiles = []
            for q in range(CHUNK):
                xt = xpool.tile([BLK, PAIR], mmdt)
                dma_xt = nc.sync.dma_start(
                    xt[:], xTr[:, ds((CHUNK * j + q) * PAIR, PAIR)]
                )
                nop_x = nc.tensor.nop()
                add_dep_helper(nop_x.ins, dma_xt.ins, reason="absorb xt dma wait")
                tiles.append(xt)
            xts[j] = tiles

        def emit_layer(t, l, tout=None):
            """One weight block: CHUNK*2 layer matmuls + relu per pair; for
            l == L, unit `tout`'s output rows ride this unit's psum tiles.
            All same-weight matmuls stay back-to-back (layer block first,
            then all riders) so each weight block is loaded exactly once."""
            j, n = divmod(t, NLOC)
            if l == 0:
                lhsT = wall_sb[:, ds(n * BLK, BLK)]
            else:
                lhsT = wall_sb[:, ds(W0_W + ((l - 1) * NLOC + n) * BLK, BLK)]
            tiles = []
            for q in range(CHUNK):
                rhs = xts[j][q] if l == 0 else hs[(t, l - 1)][q]
                ps_t, c = psum_alloc()
                for v in range(2):
                    nc.tensor.matmul(ps_t[:, ts(v, F)], lhsT, rhs[:, ts(v, F)])
                tiles.append((ps_t, c))
            if tout is not None:
                jo, no = divmod(tout, NLOC)
                lhsT_o = wall_sb[:, ds(W0_W + WH_W + no, 1)]
            res = []
            for q in range(CHUNK):
                ps_t, c = tiles[q]
                if tout is not None:
                    h3 = hs[(tout, L)][q]
                    for v in range(2):
                        nc.tensor.matmul(
                            ps_t[ds(ORow, 1), ts(v, F)],
                            lhsT_o,
                            h3[:, ts(v, F)],
                            tile_position=(0, ORow),
                        )
                ht = hpool.tile([BLK, PAIR], mmdt, tag="h")
                pointwise(ht[:], ps_t[:], True, c)
                res.append(ht)
                if tout is not None:
                    nc.sync.dma_start(
                        outT[ds(CHUNK * no + q, 1), ts(jo, PAIR)],
                        ht[ds(ORow, 1), :],
                    )
            hs[(t, l)] = res
            if l > 0:
                del hs[(t, l - 1)]
            if l == 0 and n == NLOC - 1:
                del xts[j]
            if tout is not None:
                del hs[(tout, L)]

        def emit_tail(touts):
            """Outputs of the last two units: one psum alloc per (unit, pair),
            rider row ORow each (32-aligned base required by walrus)."""
            for tout in touts:
                jo, no = divmod(tout, NLOC)
                lhsT_o = wall_sb[:, ds(W0_W + WH_W + no, 1)]
                for q in range(CHUNK):
                    ps_t, c = psum_alloc()
                    for v in range(2):
                        nc.tensor.matmul(
                            ps_t[ds(ORow, 1), ts(v, F)],
                            lhsT_o,
                            hs[(tout, L)][q][:, ts(v, F)],
                            tile_position=(0, ORow),
                        )
                    ht = hpool.tile([BLK, PAIR], mmdt, tag="h")
                    pointwise(ht[ds(ORow, 1), :], ps_t[ds(ORow, 1), :], True, c)
                    nc.sync.dma_start(
                        outT[ds(CHUNK * no + q, 1), ts(jo, PAIR)],
                        ht[ds(ORow, 1), :],
                    )
                del hs[(tout, L)]

        # Startup order: the first L0 matmuls need only the W0 block and
        # chunk 0's first x tile - issue those DMAs before the bulk of the
        # weights so the PE starts ~8us earlier.
        dma_wall_block(0, W0_W)
        prefetch_x(0)
        dma_wall_block(W0_W, W0_W + NLOC * BLK)
        dma_wall_block(W0_W + NLOC * BLK, W0_W + 2 * NLOC * BLK)
        dma_wall_block(W0_W + 2 * NLOC * BLK, WALL_W)
        for s in range(NUNIT + LAG3):
            if s % NLOC == NLOC - 2:
                prefetch_x(s // NLOC + 1)
            if s < NUNIT:
                emit_layer(s, 0)
            if LAG1 <= s < NUNIT + LAG1:
                emit_layer(s - LAG1, 1)
            if LAG2 <= s < NUNIT + LAG2:
                emit_layer(s - LAG2, 2)
            if LAG3 <= s < NUNIT + LAG3:
                t3 = s - LAG3
                tout = s - LAGO if s >= LAGO else None
                emit_layer(t3, L, tout=tout)
        emit_tail([NUNIT - 2, NUNIT - 1])
    return nc


def _split_excess_waits(bir_bytes: bytes) -> bytes:
    """BIR post-pass: walrus codegen allows at most ONE sync wait per engine
    instruction; hoist extra waits onto inserted no-update NoOps."""
    import json as _json

    d = _json.loads(bir_bytes)
    ctr = 0
    for fn in d.get("functions", []):
        for bb in fn.get("blocks", []):
            out = []
            for ins in bb.get("instructions", []):
                si = ins.get("sync_info") or {}
                ow = si.get("on_wait") or []
                if len(ow) > 1 and ins.get("engine"):
                    for w in ow[:-1]:
                        ctr += 1
                        out.append(
                            {
                                "debug": ins.get("debug", 0),
                                "engine": ins["engine"],
                                "ins": [],
                                "outs": [],
                                "name": f"WSPLIT-{ctr}",
                                "opcode": "NoOp",
                                "sync_info": {"on_update": [], "on_wait": [w]},
                            }
                        )
                    si["on_wait"] = [ow[-1]]
                out.append(ins)
            bb["instructions"] = out
    return _json.dumps(d).encode()


def install_wait_splitter():
    from concourse import bass2jax, bass_utils

    if getattr(bass_utils, "_ws_installed", False):
        return
    orig = bass_utils.compile_bir_kernel

    def patched(bir_json, tmpdir, neff_name="file.neff"):
        return orig(_split_excess_waits(bir_json), tmpdir, neff_name=neff_name)

    bass_utils.compile_bir_kernel = patched
    bass2jax.compile_bir_kernel = patched
    bass_utils._ws_installed = True


def prep_core_inputs(x, controls, W_in, b_in, W_hid, b_hid, W_out, b_out, b=B, mm_dtype=None):
    """Host-side fold + shard: list of per-core input dicts."""
    x = np.asarray(x, np.float32)
    controls = np.asarray(controls, np.float64)
    W_in = np.asarray(W_in, np.float64)
    b_in = np.asarray(b_in, np.float64)
    W_hid = np.asarray(W_hid, np.float32)
    b_hid = np.asarray(b_hid, np.float32)
    W_out = np.asarray(W_out, np.float32)
    b_out = np.asarray(b_out, np.float32)

    # controls fold: beff0[n] = controls[n] @ W_in[n, I:, :] + b_in[n]
    beff0 = (
        np.einsum("nc,nch->nh", controls, W_in[:, I:, :]) + b_in
    ).astype(np.float32)
    W_in_d = W_in[:, :I, :].astype(np.float32)  # [N, 3, H]

    # x transposed to electrode-major with ones rows, zero-padded to 128
    xT = np.ascontiguousarray(x.T).reshape(N, I, b)  # [N, 3, B]
    xTr = np.zeros((NCORES, BLK, b), np.float32)
    v = xTr[:, : 4 * NLOC, :].reshape(NCORES, NLOC, 4, b)
    v[:, :, :I, :] = xT.reshape(NCORES, NLOC, I, b)
    v[:, :, I, :] = 1.0

    npdt = mybir.dt.np(
        {"fp16": mybir.dt.float16, "bf16": mybir.dt.bfloat16, "f32r": mybir.dt.float32r}[
            mm_dtype or MM_DTYPE
        ]
    )
    in_maps = []
    for k in range(NCORES):
        g0 = k * NLOC
        wallm = np.zeros((128, WALL_W), np.float32)
        w0blk = wallm[:, :W0_W]
        whblk = wallm[:, W0_W : W0_W + WH_W]
        woblk = wallm[:, W0_W + WH_W :]
        for n in range(NLOC):
            g = g0 + n
            w0blk[4 * n : 4 * n + I, n * BLK : n * BLK + H] = W_in_d[g]
            w0blk[4 * n + I, n * BLK : n * BLK + H] = beff0[g]
            w0blk[4 * n + I, n * BLK + H] = 1.0
        for l in range(L):
            for n in range(NLOC):
                g = g0 + n
                base = (l * NLOC + n) * BLK
                whblk[:H, base : base + H] = W_hid[l, g]
                whblk[H, base : base + H] = b_hid[l, g]
                whblk[H, base + H] = 1.0
        woblk[:H, :] = W_out[g0 : g0 + NLOC].T
        woblk[H, :] = b_out[g0 : g0 + NLOC] + OFFSET
        in_maps.append(
            {
                "xTr": np.ascontiguousarray(xTr[k]).astype(npdt),
                "wall": wallm.astype(npdt),
            }
        )
    return in_maps


def run_sharded(inputs, b=B, mm_dtype=None, trace=False, **kw):
    """Build + run on the 8 cores; returns (out [b, N] fp32, BassKernelResults)."""
    from concourse import bass_utils

    install_wait_splitter()
    nc = build_nc(b=b, mm_dtype=mm_dtype)
    in_maps = prep_core_inputs(b=b, mm_dtype=mm_dtype, **inputs)
    res = bass_utils.run_bass_kernel_spmd(
        nc, in_maps, core_ids=list(range(NCORES)), trace=trace, **kw
    )
    out = np.empty((b, N), np.float32)
    nchunk = b // (CHUNK * PAIR)
    for k in range(NCORES):
        # outT row CHUNK*n+q, col j*PAIR+t  <->  batch (CHUNK*j+q)*PAIR+t
        o = res.results[k]["outT"].astype(np.float32) - OFFSET
        o = o.reshape(NLOC, CHUNK, nchunk, PAIR)
        o = o.transpose(2, 1, 3, 0).reshape(b, NLOC)
        out[:, k * NLOC : (k + 1) * NLOC] = o
    return out, res


def kernel(**inputs) -> np.ndarray:
    out, _ = run_sharded(inputs)
    return out
